# revision 54
# baseline (speedup 1.0000x reference)
"""Trainium2 Bass kernel for a GPT-style transformer block (B=2, T=2048, C=768, NH=12).

Sharding: 8 cores = 2 groups of 4 (one batch per group). Attention is
head-parallel: every core computes LN1 and q/k/v for its 3 heads over the
whole batch (2048 tokens), runs exact block-causal attention for those heads,
then applies its 192-row slice of attn_proj to get a partial projection for
each 512-token stage. A staged group-local ReduceScatter (one per 512-token
q-stage) sums the partials over the 4 cores and scatters 128-token blocks
back, giving block-cyclic token ownership (core r owns blocks r, r+4, r+8,
r+12) for the residual/LN2/MLP tail.

v2 restructuring vs the first version:
- The MLP runs per 128-token chunk, with chunk J-1's instructions enqueued
  during attention stage J, so the PE stays busy while ReduceScatters fly
  and only chunk 3's MLP trails the last collective.
- A tiny warmup ReduceScatter at kernel start absorbs the first-collective
  setup cost and inter-core skew.
- attn_proj bias is folded into the residual input on the host; q/k weights
  for the third head are packed into one 128-row stationary tile (k rows
  rebased to partition 0 by a small SBUF DMA); v bias broadcast is
  host-precomputed. LayerNorms use bn_stats/bn_aggr on vector plus a
  scalar-engine normalize (scale=rstd, bias=-mu*rstd); activation tables
  are preloaded at startup. Causal masking runs on gpsimd. PSUM: scores
  4 banks (2x2), pv 1, softmax-denominator broadcast 1, and a shared
  2-bank pool rotates the attn-proj/fc/proj tiles.
All big matmuls run bf16 (f32 PSUM accumulation); softmax is max-free
exp(s/8) with the denominator from an augmented ones column in v.
"""

import contextlib

import numpy as np
import ml_dtypes

import concourse.bacc as bacc
import concourse.bass as bass
import concourse.tile as tile
from concourse import mybir
from concourse.bass_utils import run_bass_kernel_spmd
from concourse.masks import make_identity

f32 = mybir.dt.float32
bf16 = mybir.dt.bfloat16
AF = mybir.ActivationFunctionType
OP = mybir.AluOpType
BF = ml_dtypes.bfloat16

B, T, C, NH, HD = 2, 2048, 768, 12, 64
EPS = 1e-5
N_CORES, GROUP = 8, 4
TOK = 512                      # tokens per attention stage / own tokens per core
HPC = 3                        # heads per core
KC = C // 128                  # 6 feature chunks
NT = T // 128                  # 16 token blocks
FF = 4 * C                     # 3072
MT = FF // 128                 # 24 ff chunks
FFC = FF // 512                # 6 ff column groups of 512
SCALE = 1.0 / np.sqrt(HD)
CROW = HPC * HD                # 192 ctx rows per core

_CACHE = {}


def _ap(handle, offset, pattern):
    return bass.AP(tensor=handle, offset=offset, ap=[list(p) for p in pattern])


def _build(fc_zero, pjb_zero):
    nc = bacc.Bacc("TRN2", target_bir_lowering=False, debug=False,
                   num_devices=N_CORES)

    xb_d = nc.dram_tensor("xb", [T, C], bf16, kind="ExternalInput")
    xo_d = nc.dram_tensor("xo", [4, 128, C], f32, kind="ExternalInput")
    wqkv_d = nc.dram_tensor("wqkv", [C, 3 * CROW], bf16, kind="ExternalInput")
    bqA_d = nc.dram_tensor("bqA", [128], f32, kind="ExternalInput")
    bkA_d = nc.dram_tensor("bkA", [128], f32, kind="ExternalInput")
    bqkB_d = nc.dram_tensor("bqkB", [128], f32, kind="ExternalInput")
    bvb_d = nc.dram_tensor("bvb", [128, CROW], bf16, kind="ExternalInput")
    apw_d = nc.dram_tensor("apw", [CROW, C], bf16, kind="ExternalInput")
    fcw_d = nc.dram_tensor("fcw", [C, FF], bf16, kind="ExternalInput")
    pjw_d = nc.dram_tensor("pjw", [FF, C], bf16, kind="ExternalInput")
    out_d = nc.dram_tensor("out", [4, 128, C], f32, kind="ExternalOutput")
    if not fc_zero:
        fcb_d = nc.dram_tensor("fcb_bc", [128, FF], bf16, kind="ExternalInput")
    if not pjb_zero:
        pjb_d = nc.dram_tensor("pjb_bc", [128, C], bf16, kind="ExternalInput")

    rsin = [nc.dram_tensor(f"rsin{J}", [TOK, C], bf16, kind="Internal")
            for J in range(4)]
    rsout = [nc.dram_tensor(f"rsout{J}", [128, C], bf16, kind="Internal")
             for J in range(4)]
    # warmup collective sized like the real ones so the bulk CC path (ring
    # buffers, descriptors) is hot before RS0
    warm_in = nc.dram_tensor("warm_in", [TOK, C], bf16, kind="Internal")
    warm_out = nc.dram_tensor("warm_out", [128, C], bf16, kind="Internal")

    groups = [list(range(g * GROUP, (g + 1) * GROUP)) for g in range(2)]

    with tile.TileContext(nc) as tc:
        es = contextlib.ExitStack()
        cst = es.enter_context(tc.tile_pool(name="const", bufs=1))
        wpool = es.enter_context(tc.tile_pool(name="weights", bufs=1))
        qk_pool = es.enter_context(tc.tile_pool(name="qk", bufs=1))
        va_pool = es.enter_context(tc.tile_pool(name="va", bufs=1))
        xo_pool = es.enter_context(tc.tile_pool(name="xo", bufs=1))
        x1_pool = es.enter_context(tc.tile_pool(name="x1", bufs=1))

        # ---------------- input DMAs ----------------
        ln_es = contextlib.ExitStack()
        xb_pool = ln_es.enter_context(tc.tile_pool(name="xb", bufs=1))
        hlnT_pool = ln_es.enter_context(tc.tile_pool(name="hlnT", bufs=1))

        xb = [xb_pool.tile([128, C], bf16, name=f"xb{t}") for t in range(NT)]
        for t in range(NT):
            nc.sync.dma_start(xb[t][:], xb_d.ap()[t * 128:(t + 1) * 128, :])
        # warmup collective right away: absorbs CC setup + core skew
        nc.gpsimd.collective_compute(
            "ReduceScatter", OP.add, replica_groups=groups,
            ins=[warm_in.ap().opt()], outs=[warm_out.ap().opt()])
        xo = [xo_pool.tile([128, C], f32, name=f"xo{t}") for t in range(4)]
        for t in range(4):
            nc.sync.dma_start(xo[t][:], _ap(xo_d, t * 128 * C,
                                            [[C, 128], [1, C]]))
        wq_sb = [wpool.tile([128, 3 * CROW], bf16, name=f"wq{k}")
                 for k in range(KC)]
        for k in range(KC):
            nc.sync.dma_start(
                wq_sb[k][:],
                _ap(wqkv_d, k * 128 * 3 * CROW,
                    [[3 * CROW, 128], [1, 3 * CROW]]))
        bqA = cst.tile([128, 1], f32)
        nc.sync.dma_start(bqA[:], _ap(bqA_d, 0, [[1, 128], [1, 1]]))
        bkA = cst.tile([128, 1], f32)
        nc.sync.dma_start(bkA[:], _ap(bkA_d, 0, [[1, 128], [1, 1]]))
        bqkB = cst.tile([128, 1], f32)
        nc.sync.dma_start(bqkB[:], _ap(bqkB_d, 0, [[1, 128], [1, 1]]))
        bvb = cst.tile([128, CROW], bf16)
        nc.sync.dma_start(bvb[:], bvb_d.ap())
        apwA = wpool.tile([128, C], bf16)
        nc.sync.dma_start(apwA[:], _ap(apw_d, 0, [[C, 128], [1, C]]))
        apwB = wpool.tile([64, C], bf16)
        nc.sync.dma_start(apwB[:], _ap(apw_d, 128 * C, [[C, 64], [1, C]]))
        fcw_sb = [wpool.tile([128, FF], bf16, name=f"fcw{k}")
                  for k in range(KC)]
        for k in range(KC):
            nc.sync.dma_start(
                fcw_sb[k][:], _ap(fcw_d, k * 128 * FF, [[FF, 128], [1, FF]]))
        pjw_sb = [wpool.tile([128, C], bf16, name=f"pjw{m}") for m in range(MT)]
        for m in range(MT):
            nc.sync.dma_start(
                pjw_sb[m][:], _ap(pjw_d, m * 128 * C, [[C, 128], [1, C]]))
        if not fc_zero:
            fcb_bc = wpool.tile([128, FF], bf16)
            nc.sync.dma_start(fcb_bc[:], fcb_d.ap())
        if not pjb_zero:
            pjb_bc = cst.tile([128, C], bf16)
            nc.sync.dma_start(pjb_bc[:], pjb_d.ap())

        # ---------------- constants ----------------
        tmp_es = contextlib.ExitStack()
        tmpc = tmp_es.enter_context(tc.tile_pool(name="tmpc", bufs=1))
        ident_f = tmpc.tile([128, 128], f32)
        make_identity(nc, ident_f[:])
        ident = cst.tile([128, 128], bf16)
        nc.vector.tensor_copy(ident[:], ident_f[:])
        ones_bf = cst.tile([1, 128], bf16)
        nc.vector.memset(ones_bf[:], 1.0)
        eps_t = cst.tile([128, 1], f32)
        nc.vector.memset(eps_t[:], EPS)
        # preload activation tables while the scalar engine is idle; only one
        # table stays resident, so end with Sqrt (the first one LN1 needs)
        warm_act = tmpc.tile([128, 1], f32)
        nc.scalar.activation(warm_act[:], eps_t[:], AF.Gelu_apprx_tanh)
        nc.scalar.activation(warm_act[:], eps_t[:], AF.Exp)
        nc.scalar.activation(warm_act[:], eps_t[:], AF.Sqrt)
        # causal 0/1 mask on [keys(p), q(f)]: keep where f >= p
        m01_f = tmpc.tile([128, 128], f32)
        nc.vector.memset(m01_f[:], 1.0)
        nc.gpsimd.affine_select(
            out=m01_f[:], in_=m01_f[:], compare_op=OP.is_ge, fill=0.0,
            base=0, pattern=[[1, 128]], channel_multiplier=-1)
        m01 = cst.tile([128, 128], bf16)
        nc.vector.tensor_copy(m01[:], m01_f[:])
        tmp_es.close()

        # ------------- LN1 over the whole batch (token-major) ---------------
        # bn_stats (3 x 256-wide subgroups) + bn_aggr give mean/var in one
        # vector pass; Rsqrt(var + eps) on scalar engine, batched per tg.
        stat_es = contextlib.ExitStack()
        stat_pool = stat_es.enter_context(tc.tile_pool(name="lnstat", bufs=1))
        bnst = stat_pool.tile([128, NT, 2, 6], f32)
        mv = stat_pool.tile([128, NT, 2], f32)
        sd_all = stat_pool.tile([128, NT, 2], f32)
        rstd_all = stat_pool.tile([128, NT, 2], f32)
        nmr = stat_pool.tile([128, NT], f32)
        hlnT = [hlnT_pool.tile([128, T], bf16, name=f"hlnT{k}")
                for k in range(KC)]

        # ---------------- LN1 + QKV, interleaved per token group ------------
        # LN1 is vector/scalar-bound and QKV is PE-bound; issuing QKV for
        # token group tg right after tg's transposes lets the matmuls fill
        # the PE gaps left by the stats/normalize chains of group tg+1.
        # q/k feature-major: qA [128(h0,h1), T], kA [128(h0,h1), T],
        # qkB [128, T] = rows 0:64 q(h2), 64:128 k(h2) (host-packed weights);
        # v token-major [128, 3, 65] per token block with ones column.
        qA = qk_pool.tile([128, T], bf16)
        kA = qk_pool.tile([128, T], bf16)
        qkB = qk_pool.tile([128, T], bf16)
        kB2 = qk_pool.tile([64, T], bf16)
        va = [va_pool.tile([128, HPC, HD + 1], bf16, name=f"va{t}")
              for t in range(NT)]
        for t in range(NT):
            nc.vector.memset(va[t][:, :, HD:HD + 1], 1.0)

        with (
            tc.tile_pool(name="tps", bufs=3, space="PSUM") as t_ps,
            tc.tile_pool(name="qkps", bufs=2, space="PSUM") as qk_ps,
            tc.tile_pool(name="vps", bufs=2, space="PSUM") as v_ps,
        ):
            for tg in range(4):
                tsl4 = slice(4 * tg, 4 * tg + 4)
                blocks = [[4 * tg + j for j in range(4)]]
                if tg == 0:
                    # block-at-a-time for the first group: the first
                    # transposes start ~4us earlier instead of waiting for
                    # all 4 blocks' stats
                    blocks = [[t] for t in range(4)]
                for bl in blocks:
                    bsl = slice(bl[0], bl[-1] + 1)
                    for t in bl:
                        for s in range(2):
                            nc.vector.bn_stats(
                                bnst[:, t, s, :],
                                xb[t][:, s * 384:(s + 1) * 384])
                        nc.vector.bn_aggr(mv[:, t, :], bnst[:, t, :, :])
                    # rstd = 1/sqrt(var+eps) (mean slot gets junk sqrt of
                    # mean+eps which is never read)
                    nc.scalar.activation(
                        sd_all[:, bsl, :], mv[:, bsl, :], AF.Sqrt,
                        bias=eps_t[:])
                    nc.vector.reciprocal_approx_fast(
                        rstd_all[:, bsl, :], sd_all[:, bsl, :])
                    # normalize on scalar: x*rstd + (-mu*rstd)
                    nc.vector.tensor_mul(nmr[:, bsl], mv[:, bsl, 0],
                                         rstd_all[:, bsl, 1])
                    nc.vector.tensor_scalar(nmr[:, bsl], nmr[:, bsl], -1.0,
                                            None, OP.mult)
                    for t in bl:
                        nc.scalar.activation(
                            xb[t][:], xb[t][:], AF.Identity,
                            bias=nmr[:, t:t + 1], scale=rstd_all[:, t, 1:2])
                    for k in range(KC):
                        tp = t_ps.tile([128, len(bl) * 128], bf16, name="tp")
                        for j, t in enumerate(bl):
                            nc.tensor.transpose(
                                tp[:, j * 128:(j + 1) * 128],
                                xb[t][:, k * 128:(k + 1) * 128], ident[:])
                        if k % 3 == 2:
                            nc.scalar.activation(
                                hlnT[k][:, bl[0] * 128:
                                         (bl[-1] + 1) * 128], tp[:],
                                AF.Identity)
                        else:
                            nc.vector.tensor_copy(
                                hlnT[k][:, bl[0] * 128:
                                         (bl[-1] + 1) * 128], tp[:])
                # QKV for this token group
                sl = slice(tg * 512, (tg + 1) * 512)
                for dst, boff, bias, eng in (
                    (kA, 128, bkA, "scalar"),
                    (qkB, 256, bqkB, "vector"),
                    (qA, 0, bqA, "vector"),
                ):
                    ps = qk_ps.tile([128, 512], f32, name="qkp")
                    for k in range(KC):
                        nc.tensor.matmul(
                            ps[:], wq_sb[k][:, boff:boff + 128],
                            hlnT[k][:, sl],
                            start=(k == 0), stop=(k == KC - 1))
                    if eng == "scalar":
                        nc.scalar.activation(dst[:, sl], ps[:], AF.Identity,
                                             bias=bias[:])
                    else:
                        nc.vector.tensor_scalar(dst[:, sl], ps[:], bias[:],
                                                None, OP.add)
                    if dst is qkB:
                        # rebase k(h2) rows to partition 0 for the scores
                        # stationary (matmul needs matching base partitions)
                        nc.sync.dma_start(kB2[:, sl], qkB[64:128, sl])
                for t in range(4 * tg, 4 * tg + 4):
                    vp = v_ps.tile([128, CROW], f32, name="vp")
                    for k in range(KC):
                        nc.tensor.matmul(
                            vp[:], hlnT[k][:, t * 128:(t + 1) * 128],
                            wq_sb[k][:, 384:576],
                            start=(k == 0), stop=(k == KC - 1))
                    nc.vector.tensor_add(
                        va[t][:, :, 0:HD],
                        vp[:].rearrange("p (h e) -> p h e", e=HD),
                        bvb[:].rearrange("p (h e) -> p h e", e=HD))
        stat_es.close()
        ln_es.close()

        # ------- attention (block-causal) + partial attn_proj + staged RS ----
        # interleaved with the per-chunk residual/LN2/MLP tail: chunk J-1's
        # instructions are enqueued during stage J so the PE chews on MLP
        # work while stage J's ReduceScatter is in flight.
        x1 = [x1_pool.tile([128, C], f32, name=f"x1{t}") for t in range(4)]
        qt = [(qA, 0), (qA, 64), (qkB, 0)]
        kt = [(kA, 0), (kA, 64), (kB2, 0)]
        main_es = contextlib.ExitStack()
        stpo_ps = main_es.enter_context(
            tc.tile_pool(name="stpo", bufs=2, space="PSUM"))
        pv_ps = main_es.enter_context(
            tc.tile_pool(name="pvps", bufs=2, space="PSUM"))
        mlp_ps = main_es.enter_context(
            tc.tile_pool(name="mlpps", bufs=2, space="PSUM"))
        e_pool = main_es.enter_context(tc.tile_pool(name="eps", bufs=4))
        cn_pool = main_es.enter_context(tc.tile_pool(name="cnp", bufs=2))
        ri_pool = main_es.enter_context(tc.tile_pool(name="rip", bufs=4))
        pob_pool = main_es.enter_context(tc.tile_pool(name="pob", bufs=4))
        rs_pool = main_es.enter_context(tc.tile_pool(name="rsr", bufs=2))
        ln2_pool = main_es.enter_context(tc.tile_pool(name="ln2s", bufs=1))
        h2t_pool = main_es.enter_context(tc.tile_pool(name="h2t", bufs=2))
        g_pool = main_es.enter_context(tc.tile_pool(name="gp", bufs=2))
        oo_pool = main_es.enter_context(tc.tile_pool(name="oop", bufs=2))

        bn2 = ln2_pool.tile([128, 4, 2, 6], f32)
        mv2 = ln2_pool.tile([128, 4, 2], f32)
        sd2 = ln2_pool.tile([128, 4, 2], f32)
        rstd2 = ln2_pool.tile([128, 4, 2], f32)
        nmr2 = ln2_pool.tile([128, 4], f32)

        def attn_stage(J, mid=None):
            nkb = 4 * (J + 1)
            kbs = list(range(nkb))
            groups2 = [kbs[i:i + 2] for i in range(0, nkb, 2)]
            cnA = cn_pool.tile([128, TOK], bf16, name="cnA")
            cnB = cn_pool.tile([64, TOK], bf16, name="cnB")
            for h in range(HPC):
                if h == 1 and mid is not None:
                    # enqueue the previous chunk's MLP here: its collective
                    # has certainly landed, and head-0 score work above fills
                    # the engines while the chunk's LN2 chain spins up
                    mid()
                qtile, qoff_p = qt[h]
                ktile, koff_p = kt[h]
                pv = pv_ps.tile([HD + 1, TOK], f32, name="pv")

                def scores(grp):
                    sT = stpo_ps.tile([128, 2, TOK], f32, name="sT")
                    ep = e_pool.tile([128, 2, TOK], bf16, name="ep")
                    for j, kb in enumerate(grp):
                        qo = max(0, kb * 128 - J * TOK)
                        nc.tensor.matmul(
                            sT[:, j, qo:TOK],
                            ktile[koff_p:koff_p + HD,
                                  kb * 128:(kb + 1) * 128],
                            qtile[qoff_p:qoff_p + HD,
                                  J * TOK + qo:(J + 1) * TOK],
                            start=True, stop=True)
                    nc.scalar.activation(ep[:, 0:len(grp), :],
                                         sT[:, 0:len(grp), :],
                                         AF.Exp, bias=0.0, scale=SCALE)
                    for j, kb in enumerate(grp):
                        qo = max(0, kb * 128 - J * TOK)
                        if kb >= 4 * J:
                            nc.gpsimd.tensor_mul(
                                ep[:, j, qo:qo + 128],
                                ep[:, j, qo:qo + 128], m01[:])
                    return ep

                def pvacc(grp, ep):
                    for j, kb in enumerate(grp):
                        qo = max(0, kb * 128 - J * TOK)
                        nc.tensor.matmul(
                            pv[:, qo:TOK], va[kb][:, h, :],
                            ep[:, j, qo:TOK],
                            start=(kb == 0), stop=(kb == nkb - 1))

                pend = None
                for grp in groups2:
                    ep = scores(grp)
                    if pend is not None:
                        pvacc(*pend)
                    pend = (grp, ep)
                pvacc(*pend)
                # normalize: rinv = 1/den, broadcast over 64 rows via PE
                # (den must be staged to SBUF first: reciprocal_approx_fast
                # is a multi-op DVE sequence and cannot read PSUM directly)
                dsb = ri_pool.tile([1, TOK], f32, name="dsb")
                nc.vector.tensor_copy(dsb[:], pv[HD:HD + 1, :])
                # stage the unnormalized ctx out of PSUM first, then reuse
                # pv's rows 0:64 for the rinv broadcast — saves a PSUM bank
                # so pv can double-buffer across heads
                cu = ri_pool.tile([64, TOK], bf16, name="cu")
                nc.vector.tensor_copy(cu[:], pv[0:HD, :])
                rinv = ri_pool.tile([1, TOK], f32, name="rinv")
                nc.vector.reciprocal_approx_fast(rinv[:], dsb[:])
                rinvb = ri_pool.tile([1, TOK], bf16, name="rinvb")
                nc.vector.tensor_copy(rinvb[:], rinv[:])
                nc.tensor.matmul(pv[0:HD, :], ones_bf[0:1, 0:64], rinvb[:],
                                 start=True, stop=True)
                cdst = cnA[0:64, :] if h == 0 else (
                    cnA[64:128, :] if h == 1 else cnB[:])
                nc.vector.tensor_mul(cdst, cu[:], pv[0:HD, :])
            # partial attn_proj for this stage's 512 tokens (bias folded
            # into xo on the host)
            for ts in range(4):
                tsl = slice(ts * 128, (ts + 1) * 128)
                po_sb = pob_pool.tile([128, C], bf16, name="po_sb")
                for c0, c1 in ((0, 512), (512, C)):
                    po = mlp_ps.tile([128, c1 - c0], f32, name="po",
                                     tag="mlp")
                    nc.tensor.matmul(po[:], cnA[:, tsl], apwA[:, c0:c1],
                                     start=True, stop=False)
                    nc.tensor.matmul(po[:], cnB[:, tsl], apwB[:, c0:c1],
                                     start=False, stop=True)
                    nc.vector.tensor_copy(po_sb[:, c0:c1], po[:])
                nc.sync.dma_start(
                    rsin[J].ap()[ts * 128:(ts + 1) * 128, :], po_sb[:])
            nc.gpsimd.collective_compute(
                "ReduceScatter", OP.add, replica_groups=groups,
                ins=[rsin[J].ap().opt()], outs=[rsout[J].ap().opt()])

        def mlp_chunk(J):
            # residual + LN2 for the 128 owned tokens of stage J
            rso = rs_pool.tile([128, C], bf16, name="rso")
            nc.sync.dma_start(rso[:], rsout[J].ap())
            nc.vector.tensor_add(x1[J][:], rso[:], xo[J][:])
            for s in range(2):
                nc.vector.bn_stats(bn2[:, J, s, :],
                                   x1[J][:, s * 384:(s + 1) * 384])
            nc.vector.bn_aggr(mv2[:, J, :], bn2[:, J, :, :])
            nc.scalar.activation(sd2[:, J, :], mv2[:, J, :],
                                 AF.Sqrt, bias=eps_t[:])
            nc.vector.reciprocal_approx_fast(rstd2[:, J, :], sd2[:, J, :])
            nc.vector.tensor_mul(nmr2[:, J:J + 1], mv2[:, J, 0:1],
                                 rstd2[:, J, 1:2])
            nc.vector.tensor_scalar(nmr2[:, J:J + 1], nmr2[:, J:J + 1], -1.0,
                                    None, OP.mult)
            h2c = rs_pool.tile([128, C], bf16, name="h2c")
            nc.scalar.activation(h2c[:], x1[J][:], AF.Identity,
                                 bias=nmr2[:, J:J + 1],
                                 scale=rstd2[:, J, 1:2])
            # transpose h2c -> h2T [128 feat, 128 tok] (6 feature chunks)
            h2T = h2t_pool.tile([128, KC, 128], bf16, name="h2T")
            for half, n in ((0, 4), (4, 2)):
                tp2 = mlp_ps.tile([128, n * 128], bf16, name="tp2",
                                  tag="mlp")
                for k in range(n):
                    nc.tensor.transpose(
                        tp2[:, k * 128:(k + 1) * 128],
                        h2c[:, (half + k) * 128:(half + k + 1) * 128],
                        ident[:])
                nc.vector.tensor_copy(
                    h2T[:, half:half + n, :].rearrange("p a b -> p (a b)"),
                    tp2[:])
            # fc, token-major: gt [128 tok, 512 ff] per ff group, gelu,
            # then transpose to ff-major g for the proj stationary
            gf = g_pool.tile([128, MT, 128], bf16, name="gf")
            gt_sb = [None] * FFC
            for i in range(FFC):
                gp = mlp_ps.tile([128, 512], f32, name="gp", tag="mlp")
                fsl = slice(i * 512, (i + 1) * 512)
                for k in range(KC):
                    nc.tensor.matmul(gp[:], h2T[:, k, :],
                                     fcw_sb[k][:, fsl],
                                     start=(k == 0), stop=(k == KC - 1))
                gt = g_pool.tile([128, 512], bf16, name="gt", bufs=3)
                if fc_zero:
                    nc.scalar.activation(gt[:], gp[:], AF.Gelu_apprx_tanh,
                                         bias=0.0)
                else:
                    nc.vector.tensor_add(gp[:], gp[:], fcb_bc[:, fsl])
                    nc.scalar.activation(gt[:], gp[:], AF.Gelu_apprx_tanh,
                                         bias=0.0)
                gt_sb[i] = gt
                if i > 0:
                    tr_g(gf, gt_sb[i - 1], i - 1)
            tr_g(gf, gt_sb[FFC - 1], FFC - 1)
            # proj + residual (+ pjb) -> out, stored per half so the first
            # half's DMA overlaps the second half's matmuls
            oo = oo_pool.tile([128, C], f32, name="oo")
            for c0, c1 in ((0, 512), (512, C)):
                pr = mlp_ps.tile([128, c1 - c0], f32, name="pr", tag="mlp")
                for m in range(MT):
                    nc.tensor.matmul(pr[:], gf[:, m, :],
                                     pjw_sb[m][:, c0:c1],
                                     start=(m == 0), stop=(m == MT - 1))
                nc.vector.tensor_add(oo[:, c0:c1], pr[:], x1[J][:, c0:c1])
                if not pjb_zero:
                    nc.vector.tensor_add(oo[:, c0:c1], oo[:, c0:c1],
                                         pjb_bc[:, c0:c1])
                nc.sync.dma_start(
                    _ap(out_d, J * 128 * C + c0, [[C, 128], [1, c1 - c0]]),
                    oo[:, c0:c1])

        def tr_g(gf, gt, i):
            tpg = mlp_ps.tile([128, 512], bf16, name="tpg", tag="mlp")
            for k in range(4):
                nc.tensor.transpose(tpg[:, k * 128:(k + 1) * 128],
                                    gt[:, k * 128:(k + 1) * 128], ident[:])
            nc.vector.tensor_copy(
                gf[:, 4 * i:4 * i + 4, :].rearrange("p a b -> p (a b)"),
                tpg[:])

        # chunk 0 consumes RS0, which can land late: defer it to stage 2's
        # head boundary so stage-2 scores fill the RS0 wait. Later chunks
        # follow the same pattern one stage after their ReduceScatter.
        attn_stage(0)
        attn_stage(1)
        attn_stage(2, mid=lambda: mlp_chunk(0))
        attn_stage(3, mid=lambda: mlp_chunk(1))
        mlp_chunk(2)
        mlp_chunk(3)
        main_es.close()
        es.close()

    nc.compile()
    return nc


def _prepare(inputs):
    """Host-side weight prep shared by kernel() and test harness."""
    x = np.asarray(inputs["x"], np.float32)
    ln1s = np.asarray(inputs["ln1_scale"], np.float32)
    ln1b = np.asarray(inputs["ln1_bias"], np.float32)
    ln2s = np.asarray(inputs["ln2_scale"], np.float32)
    ln2b = np.asarray(inputs["ln2_bias"], np.float32)
    wqkv = np.asarray(inputs["wqkv"], np.float32)          # [C,3,NH,HD]
    bqkv = np.asarray(inputs["bqkv"], np.float32)          # [3,NH,HD]
    apw = np.asarray(inputs["attn_proj_w"], np.float32)
    apb = np.asarray(inputs["attn_proj_b"], np.float32)
    fcw = np.asarray(inputs["fc_w"], np.float32)
    fcb = np.asarray(inputs["fc_b"], np.float32)
    pjw = np.asarray(inputs["proj_w"], np.float32)
    pjb = np.asarray(inputs["proj_b"], np.float32)

    # fold LN1 affine into wqkv/bqkv, LN2 affine into fcw/fcb (exact)
    wqkv_f = wqkv * ln1s[:, None, None, None]
    bqkv_f = bqkv + np.einsum("c,cshd->shd", ln1b, wqkv)
    fcw_f = fcw * ln2s[:, None]
    fcb_f = fcb + ln2b @ fcw

    fc_zero = bool(np.all(fcb_f == 0.0))
    pjb_zero = bool(np.all(pjb == 0.0))

    shared = {
        "fcw": np.ascontiguousarray(fcw_f).astype(BF),
        "pjw": pjw.astype(BF),
    }
    if not fc_zero:
        shared["fcb_bc"] = np.ascontiguousarray(
            np.broadcast_to(fcb_f[None, :], (128, FF))).astype(BF)
    if not pjb_zero:
        shared["pjb_bc"] = np.ascontiguousarray(
            np.broadcast_to(pjb[None, :], (128, C))).astype(BF)
    in_maps = []
    for core in range(N_CORES):
        b, r = divmod(core, GROUP)
        hs = slice(HPC * r, HPC * (r + 1))
        wq = wqkv_f[:, 0, hs, :].reshape(C, CROW)
        wk = wqkv_f[:, 1, hs, :].reshape(C, CROW)
        wv = wqkv_f[:, 2, hs, :].reshape(C, CROW)
        bq = bqkv_f[0, hs, :].reshape(CROW)
        bk = bqkv_f[1, hs, :].reshape(CROW)
        bv = bqkv_f[2, hs, :].reshape(CROW)
        m = dict(shared)
        # packed: [qA(128) | kA(128) | q_h2(64)+k_h2(64) | v(192)]
        m["wqkv"] = np.ascontiguousarray(np.concatenate(
            [wq[:, :128], wk[:, :128], wq[:, 128:], wk[:, 128:], wv],
            axis=1)).astype(BF)
        m["bqA"] = np.ascontiguousarray(bq[:128]).astype(np.float32)
        m["bkA"] = np.ascontiguousarray(bk[:128]).astype(np.float32)
        m["bqkB"] = np.ascontiguousarray(
            np.concatenate([bq[128:], bk[128:]])).astype(np.float32)
        m["bvb"] = np.ascontiguousarray(
            np.broadcast_to(bv[None, :], (128, CROW))).astype(BF)
        m["apw"] = np.ascontiguousarray(
            apw[CROW * r:CROW * (r + 1), :]).astype(BF)
        m["xb"] = np.ascontiguousarray(x[b]).astype(BF)
        # attn_proj bias folded into the residual input (exact)
        m["xo"] = np.ascontiguousarray(
            x[b].reshape(NT, 128, C)[r::GROUP] + apb[None, None, :])
        in_maps.append(m)
    return in_maps, fc_zero, pjb_zero


def kernel(x, mask, ln1_scale, ln1_bias, wqkv, bqkv, attn_proj_w, attn_proj_b,
           ln2_scale, ln2_bias, fc_w, fc_b, proj_w, proj_b):
    in_maps, fc_zero, pjb_zero = _prepare(dict(
        x=x, ln1_scale=ln1_scale, ln1_bias=ln1_bias, wqkv=wqkv, bqkv=bqkv,
        attn_proj_w=attn_proj_w, attn_proj_b=attn_proj_b,
        ln2_scale=ln2_scale, ln2_bias=ln2_bias, fc_w=fc_w, fc_b=fc_b,
        proj_w=proj_w, proj_b=proj_b))
    key = (fc_zero, pjb_zero)
    if _CACHE.get("key") != key:
        _CACHE["nc"] = _build(fc_zero, pjb_zero)
        _CACHE["key"] = key
    nc = _CACHE["nc"]
    res = run_bass_kernel_spmd(nc, in_maps, list(range(N_CORES)))
    _CACHE["last_result"] = res
    return assemble(res.results)


def assemble(results):
    """Assemble per-core 'out' results into the full [B, T, C] output."""
    out = np.empty((B, T, C), dtype=np.float32)
    for core in range(N_CORES):
        b, r = divmod(core, GROUP)
        out[b].reshape(NT, 128, C)[r::GROUP] = results[core]["out"]
    return out


# revision 55
# speedup vs baseline: 1.0412x; 1.0412x over previous
"""Trainium2 Bass kernel for a GPT-style transformer block (B=2, T=2048, C=768, NH=12).

Sharding: 8 cores = 2 groups of 4 (one batch per group). Attention is
head-parallel: every core computes LN1 and q/k/v for its 3 heads over the
whole batch (2048 tokens), runs exact block-causal attention for those heads,
then applies its 192-row slice of attn_proj to get a partial projection for
each 512-token stage. A staged group-local ReduceScatter (one per 512-token
q-stage) sums the partials over the 4 cores and scatters 128-token blocks
back, giving block-cyclic token ownership (core r owns blocks r, r+4, r+8,
r+12) for the residual/LN2/MLP tail.

v2 restructuring vs the first version:
- The MLP runs per 128-token chunk, with chunk J-1's instructions enqueued
  during attention stage J, so the PE stays busy while ReduceScatters fly
  and only chunk 3's MLP trails the last collective.
- A tiny warmup ReduceScatter at kernel start absorbs the first-collective
  setup cost and inter-core skew.
- attn_proj bias is folded into the residual input on the host; q/k weights
  for the third head are packed into one 128-row stationary tile (k rows
  rebased to partition 0 by a small SBUF DMA); v bias broadcast is
  host-precomputed. LayerNorms use bn_stats/bn_aggr on vector plus a
  scalar-engine normalize (scale=rstd, bias=-mu*rstd); activation tables
  are preloaded at startup. Causal masking runs on gpsimd. PSUM: scores
  4 banks (2x2), pv 1, softmax-denominator broadcast 1, and a shared
  2-bank pool rotates the attn-proj/fc/proj tiles.
All big matmuls run bf16 (f32 PSUM accumulation); softmax is max-free
exp(s/8) with the denominator from an augmented ones column in v.
"""

import contextlib

import numpy as np
import ml_dtypes

import concourse.bacc as bacc
import concourse.bass as bass
import concourse.tile as tile
from concourse import mybir
from concourse.bass_utils import run_bass_kernel_spmd
from concourse.masks import make_identity

f32 = mybir.dt.float32
bf16 = mybir.dt.bfloat16
AF = mybir.ActivationFunctionType
OP = mybir.AluOpType
BF = ml_dtypes.bfloat16

B, T, C, NH, HD = 2, 2048, 768, 12, 64
EPS = 1e-5
N_CORES, GROUP = 8, 4
TOK = 512                      # tokens per attention stage / own tokens per core
HPC = 3                        # heads per core
KC = C // 128                  # 6 feature chunks
NT = T // 128                  # 16 token blocks
FF = 4 * C                     # 3072
MT = FF // 128                 # 24 ff chunks
FFC = FF // 512                # 6 ff column groups of 512
SCALE = 1.0 / np.sqrt(HD)
CROW = HPC * HD                # 192 ctx rows per core

_CACHE = {}


def _ap(handle, offset, pattern):
    return bass.AP(tensor=handle, offset=offset, ap=[list(p) for p in pattern])


def _build(fc_zero, pjb_zero):
    nc = bacc.Bacc("TRN2", target_bir_lowering=False, debug=False,
                   num_devices=N_CORES)

    xb_d = nc.dram_tensor("xb", [T, C], bf16, kind="ExternalInput")
    xo_d = nc.dram_tensor("xo", [4, 128, C], f32, kind="ExternalInput")
    wqkv_d = nc.dram_tensor("wqkv", [C, 3 * CROW], bf16, kind="ExternalInput")
    bqA_d = nc.dram_tensor("bqA", [128], f32, kind="ExternalInput")
    bkA_d = nc.dram_tensor("bkA", [128], f32, kind="ExternalInput")
    bqkB_d = nc.dram_tensor("bqkB", [128], f32, kind="ExternalInput")
    bvb_d = nc.dram_tensor("bvb", [128, CROW], bf16, kind="ExternalInput")
    apw_d = nc.dram_tensor("apw", [CROW, C], bf16, kind="ExternalInput")
    fcw_d = nc.dram_tensor("fcw", [C, FF], bf16, kind="ExternalInput")
    pjw_d = nc.dram_tensor("pjw", [FF, C], bf16, kind="ExternalInput")
    out_d = nc.dram_tensor("out", [4, 128, C], f32, kind="ExternalOutput")
    if not fc_zero:
        fcb_d = nc.dram_tensor("fcb_bc", [128, FF], bf16, kind="ExternalInput")
    if not pjb_zero:
        pjb_d = nc.dram_tensor("pjb_bc", [128, C], bf16, kind="ExternalInput")

    rsin = [nc.dram_tensor(f"rsin{J}", [TOK, C], bf16, kind="Internal")
            for J in range(4)]
    rsout = [nc.dram_tensor(f"rsout{J}", [128, C], bf16, kind="Internal")
             for J in range(4)]
    # warmup collective sized like the real ones so the bulk CC path (ring
    # buffers, descriptors) is hot before RS0
    warm_in = nc.dram_tensor("warm_in", [TOK, C], bf16, kind="Internal")
    warm_out = nc.dram_tensor("warm_out", [128, C], bf16, kind="Internal")

    groups = [list(range(g * GROUP, (g + 1) * GROUP)) for g in range(2)]

    with tile.TileContext(nc) as tc:
        es = contextlib.ExitStack()
        cst = es.enter_context(tc.tile_pool(name="const", bufs=1))
        wpool = es.enter_context(tc.tile_pool(name="weights", bufs=1))
        qk_pool = es.enter_context(tc.tile_pool(name="qk", bufs=1))
        va_pool = es.enter_context(tc.tile_pool(name="va", bufs=1))
        xo_pool = es.enter_context(tc.tile_pool(name="xo", bufs=1))
        x1_pool = es.enter_context(tc.tile_pool(name="x1", bufs=1))

        # ---------------- input DMAs ----------------
        ln_es = contextlib.ExitStack()
        xb_pool = ln_es.enter_context(tc.tile_pool(name="xb", bufs=1))
        hlnT_pool = ln_es.enter_context(tc.tile_pool(name="hlnT", bufs=1))

        xb = [xb_pool.tile([128, C], bf16, name=f"xb{t}") for t in range(NT)]
        for t in range(NT):
            nc.sync.dma_start(xb[t][:], xb_d.ap()[t * 128:(t + 1) * 128, :])
        # warmup collective right away: absorbs CC setup + core skew
        nc.gpsimd.collective_compute(
            "ReduceScatter", OP.add, replica_groups=groups,
            ins=[warm_in.ap().opt()], outs=[warm_out.ap().opt()])
        xo = [xo_pool.tile([128, C], f32, name=f"xo{t}") for t in range(4)]
        for t in range(4):
            nc.sync.dma_start(xo[t][:], _ap(xo_d, t * 128 * C,
                                            [[C, 128], [1, C]]))
        wq_sb = [wpool.tile([128, 3 * CROW], bf16, name=f"wq{k}")
                 for k in range(KC)]
        for k in range(KC):
            nc.sync.dma_start(
                wq_sb[k][:],
                _ap(wqkv_d, k * 128 * 3 * CROW,
                    [[3 * CROW, 128], [1, 3 * CROW]]))
        bqA = cst.tile([128, 1], f32)
        nc.sync.dma_start(bqA[:], _ap(bqA_d, 0, [[1, 128], [1, 1]]))
        bkA = cst.tile([128, 1], f32)
        nc.sync.dma_start(bkA[:], _ap(bkA_d, 0, [[1, 128], [1, 1]]))
        bqkB = cst.tile([128, 1], f32)
        nc.sync.dma_start(bqkB[:], _ap(bqkB_d, 0, [[1, 128], [1, 1]]))
        bvb = cst.tile([128, CROW], bf16)
        nc.sync.dma_start(bvb[:], bvb_d.ap())
        apwA = wpool.tile([128, C], bf16)
        nc.sync.dma_start(apwA[:], _ap(apw_d, 0, [[C, 128], [1, C]]))
        apwB = wpool.tile([64, C], bf16)
        nc.sync.dma_start(apwB[:], _ap(apw_d, 128 * C, [[C, 64], [1, C]]))
        fcw_sb = [wpool.tile([128, FF], bf16, name=f"fcw{k}")
                  for k in range(KC)]
        for k in range(KC):
            nc.sync.dma_start(
                fcw_sb[k][:], _ap(fcw_d, k * 128 * FF, [[FF, 128], [1, FF]]))
        pjw_sb = [wpool.tile([128, C], bf16, name=f"pjw{m}") for m in range(MT)]
        for m in range(MT):
            nc.sync.dma_start(
                pjw_sb[m][:], _ap(pjw_d, m * 128 * C, [[C, 128], [1, C]]))
        if not fc_zero:
            fcb_bc = wpool.tile([128, FF], bf16)
            nc.sync.dma_start(fcb_bc[:], fcb_d.ap())
        if not pjb_zero:
            pjb_bc = cst.tile([128, C], bf16)
            nc.sync.dma_start(pjb_bc[:], pjb_d.ap())

        # ---------------- constants ----------------
        tmp_es = contextlib.ExitStack()
        tmpc = tmp_es.enter_context(tc.tile_pool(name="tmpc", bufs=1))
        ident_f = tmpc.tile([128, 128], f32)
        make_identity(nc, ident_f[:])
        ident = cst.tile([128, 128], bf16)
        nc.vector.tensor_copy(ident[:], ident_f[:])
        ones_bf = cst.tile([1, 128], bf16)
        nc.vector.memset(ones_bf[:], 1.0)
        eps_t = cst.tile([128, 1], f32)
        nc.vector.memset(eps_t[:], EPS)
        # preload activation tables while the scalar engine is idle; only one
        # table stays resident, so end with Sqrt (the first one LN1 needs)
        warm_act = tmpc.tile([128, 1], f32)
        nc.scalar.activation(warm_act[:], eps_t[:], AF.Gelu_apprx_tanh)
        nc.scalar.activation(warm_act[:], eps_t[:], AF.Exp)
        nc.scalar.activation(warm_act[:], eps_t[:], AF.Sqrt)
        # causal 0/1 mask on [keys(p), q(f)]: keep where f >= p
        m01_f = tmpc.tile([128, 128], f32)
        nc.vector.memset(m01_f[:], 1.0)
        nc.gpsimd.affine_select(
            out=m01_f[:], in_=m01_f[:], compare_op=OP.is_ge, fill=0.0,
            base=0, pattern=[[1, 128]], channel_multiplier=-1)
        m01 = cst.tile([128, 128], bf16)
        nc.vector.tensor_copy(m01[:], m01_f[:])
        tmp_es.close()

        # ------------- LN1 over the whole batch (token-major) ---------------
        # bn_stats (3 x 256-wide subgroups) + bn_aggr give mean/var in one
        # vector pass; Rsqrt(var + eps) on scalar engine, batched per tg.
        stat_es = contextlib.ExitStack()
        stat_pool = stat_es.enter_context(tc.tile_pool(name="lnstat", bufs=1))
        bnst = stat_pool.tile([128, NT, 2, 6], f32)
        mv = stat_pool.tile([128, NT, 2], f32)
        sd_all = stat_pool.tile([128, NT, 2], f32)
        rstd_all = stat_pool.tile([128, NT, 2], f32)
        nmr = stat_pool.tile([128, NT], f32)
        hlnT = [hlnT_pool.tile([128, T], bf16, name=f"hlnT{k}")
                for k in range(KC)]

        # ---------------- LN1 + QKV, interleaved per token group ------------
        # LN1 is vector/scalar-bound and QKV is PE-bound; issuing QKV for
        # token group tg right after tg's transposes lets the matmuls fill
        # the PE gaps left by the stats/normalize chains of group tg+1.
        # q/k feature-major: qA [128(h0,h1), T], kA [128(h0,h1), T],
        # qkB [128, T] = rows 0:64 q(h2), 64:128 k(h2) (host-packed weights);
        # v token-major [128, 3, 65] per token block with ones column.
        qA = qk_pool.tile([128, T], bf16)
        kA = qk_pool.tile([128, T], bf16)
        qkB = qk_pool.tile([128, T], bf16)
        kB2 = qk_pool.tile([64, T], bf16)
        va = [va_pool.tile([128, HPC, HD + 1], bf16, name=f"va{t}")
              for t in range(NT)]
        for t in range(NT):
            nc.vector.memset(va[t][:, :, HD:HD + 1], 1.0)

        with (
            tc.tile_pool(name="tps", bufs=3, space="PSUM") as t_ps,
            tc.tile_pool(name="qkps", bufs=2, space="PSUM") as qk_ps,
            tc.tile_pool(name="vps", bufs=2, space="PSUM") as v_ps,
        ):
            for tg in range(4):
                tsl4 = slice(4 * tg, 4 * tg + 4)
                for j in range(4):
                    t = 4 * tg + j
                    for s in range(2):
                        nc.vector.bn_stats(
                            bnst[:, t, s, :],
                            xb[t][:, s * 384:(s + 1) * 384])
                    nc.vector.bn_aggr(mv[:, t, :], bnst[:, t, :, :])
                # rstd = 1/sqrt(var+eps) for the 4 blocks (mean slot gets
                # junk sqrt of mean+eps which is never read)
                nc.scalar.activation(
                    sd_all[:, tsl4, :], mv[:, tsl4, :], AF.Sqrt,
                    bias=eps_t[:])
                nc.vector.reciprocal_approx_fast(
                    rstd_all[:, tsl4, :], sd_all[:, tsl4, :])
                # normalize on scalar: x*rstd + (-mu*rstd)
                nc.vector.tensor_mul(nmr[:, tsl4], mv[:, tsl4, 0],
                                     rstd_all[:, tsl4, 1])
                nc.vector.tensor_scalar(nmr[:, tsl4], nmr[:, tsl4], -1.0,
                                        None, OP.mult)
                for j in range(4):
                    t = 4 * tg + j
                    nc.scalar.activation(
                        xb[t][:], xb[t][:], AF.Identity,
                        bias=nmr[:, t:t + 1], scale=rstd_all[:, t, 1:2])
                for k in range(KC):
                    tp = t_ps.tile([128, 512], bf16, name="tp")
                    for j in range(4):
                        t = 4 * tg + j
                        nc.tensor.transpose(
                            tp[:, j * 128:(j + 1) * 128],
                            xb[t][:, k * 128:(k + 1) * 128], ident[:])
                    if k % 3 == 2:
                        nc.scalar.activation(
                            hlnT[k][:, tg * 512:(tg + 1) * 512], tp[:],
                            AF.Identity)
                    else:
                        nc.vector.tensor_copy(
                            hlnT[k][:, tg * 512:(tg + 1) * 512], tp[:])
                # QKV for this token group
                sl = slice(tg * 512, (tg + 1) * 512)
                for dst, boff, bias, eng in (
                    (kA, 128, bkA, "scalar"),
                    (qkB, 256, bqkB, "vector"),
                    (qA, 0, bqA, "vector"),
                ):
                    ps = qk_ps.tile([128, 512], f32, name="qkp")
                    for k in range(KC):
                        nc.tensor.matmul(
                            ps[:], wq_sb[k][:, boff:boff + 128],
                            hlnT[k][:, sl],
                            start=(k == 0), stop=(k == KC - 1))
                    if eng == "scalar":
                        nc.scalar.activation(dst[:, sl], ps[:], AF.Identity,
                                             bias=bias[:])
                    else:
                        nc.vector.tensor_scalar(dst[:, sl], ps[:], bias[:],
                                                None, OP.add)
                    if dst is qkB:
                        # rebase k(h2) rows to partition 0 for the scores
                        # stationary (matmul needs matching base partitions)
                        nc.sync.dma_start(kB2[:, sl], qkB[64:128, sl])
                for t in range(4 * tg, 4 * tg + 4):
                    vp = v_ps.tile([128, CROW], f32, name="vp")
                    for k in range(KC):
                        nc.tensor.matmul(
                            vp[:], hlnT[k][:, t * 128:(t + 1) * 128],
                            wq_sb[k][:, 384:576],
                            start=(k == 0), stop=(k == KC - 1))
                    nc.vector.tensor_add(
                        va[t][:, :, 0:HD],
                        vp[:].rearrange("p (h e) -> p h e", e=HD),
                        bvb[:].rearrange("p (h e) -> p h e", e=HD))
        stat_es.close()
        ln_es.close()

        # ------- attention (block-causal) + partial attn_proj + staged RS ----
        # interleaved with the per-chunk residual/LN2/MLP tail: chunk J-1's
        # instructions are enqueued during stage J so the PE chews on MLP
        # work while stage J's ReduceScatter is in flight.
        x1 = [x1_pool.tile([128, C], f32, name=f"x1{t}") for t in range(4)]
        qt = [(qA, 0), (qA, 64), (qkB, 0)]
        kt = [(kA, 0), (kA, 64), (kB2, 0)]
        main_es = contextlib.ExitStack()
        stpo_ps = main_es.enter_context(
            tc.tile_pool(name="stpo", bufs=2, space="PSUM"))
        pv_ps = main_es.enter_context(
            tc.tile_pool(name="pvps", bufs=2, space="PSUM"))
        mlp_ps = main_es.enter_context(
            tc.tile_pool(name="mlpps", bufs=2, space="PSUM"))
        e_pool = main_es.enter_context(tc.tile_pool(name="eps", bufs=4))
        cn_pool = main_es.enter_context(tc.tile_pool(name="cnp", bufs=2))
        ri_pool = main_es.enter_context(tc.tile_pool(name="rip", bufs=4))
        pob_pool = main_es.enter_context(tc.tile_pool(name="pob", bufs=4))
        rs_pool = main_es.enter_context(tc.tile_pool(name="rsr", bufs=2))
        ln2_pool = main_es.enter_context(tc.tile_pool(name="ln2s", bufs=1))
        h2t_pool = main_es.enter_context(tc.tile_pool(name="h2t", bufs=2))
        g_pool = main_es.enter_context(tc.tile_pool(name="gp", bufs=2))
        oo_pool = main_es.enter_context(tc.tile_pool(name="oop", bufs=2))

        bn2 = ln2_pool.tile([128, 4, 2, 6], f32)
        mv2 = ln2_pool.tile([128, 4, 2], f32)
        sd2 = ln2_pool.tile([128, 4, 2], f32)
        rstd2 = ln2_pool.tile([128, 4, 2], f32)
        nmr2 = ln2_pool.tile([128, 4], f32)

        def attn_stage(J, mid=None):
            nkb = 4 * (J + 1)
            kbs = list(range(nkb))
            groups2 = [kbs[i:i + 2] for i in range(0, nkb, 2)]
            cnA = cn_pool.tile([128, TOK], bf16, name="cnA")
            cnB = cn_pool.tile([64, TOK], bf16, name="cnB")
            for h in range(HPC):
                if h == 1 and mid is not None:
                    # enqueue the previous chunk's MLP here: its collective
                    # has certainly landed, and head-0 score work above fills
                    # the engines while the chunk's LN2 chain spins up
                    mid()
                qtile, qoff_p = qt[h]
                ktile, koff_p = kt[h]
                pv = pv_ps.tile([HD + 1, TOK], f32, name="pv")

                def scores(grp):
                    sT = stpo_ps.tile([128, 2, TOK], f32, name="sT")
                    ep = e_pool.tile([128, 2, TOK], bf16, name="ep")
                    for j, kb in enumerate(grp):
                        qo = max(0, kb * 128 - J * TOK)
                        nc.tensor.matmul(
                            sT[:, j, qo:TOK],
                            ktile[koff_p:koff_p + HD,
                                  kb * 128:(kb + 1) * 128],
                            qtile[qoff_p:qoff_p + HD,
                                  J * TOK + qo:(J + 1) * TOK],
                            start=True, stop=True)
                    nc.scalar.activation(ep[:, 0:len(grp), :],
                                         sT[:, 0:len(grp), :],
                                         AF.Exp, bias=0.0, scale=SCALE)
                    for j, kb in enumerate(grp):
                        qo = max(0, kb * 128 - J * TOK)
                        if kb >= 4 * J:
                            nc.gpsimd.tensor_mul(
                                ep[:, j, qo:qo + 128],
                                ep[:, j, qo:qo + 128], m01[:])
                    return ep

                def pvacc(grp, ep):
                    for j, kb in enumerate(grp):
                        qo = max(0, kb * 128 - J * TOK)
                        nc.tensor.matmul(
                            pv[:, qo:TOK], va[kb][:, h, :],
                            ep[:, j, qo:TOK],
                            start=(kb == 0), stop=(kb == nkb - 1))

                pend = None
                for grp in groups2:
                    ep = scores(grp)
                    if pend is not None:
                        pvacc(*pend)
                    pend = (grp, ep)
                pvacc(*pend)
                # normalize: rinv = 1/den, broadcast over 64 rows via PE
                # (den must be staged to SBUF first: reciprocal_approx_fast
                # is a multi-op DVE sequence and cannot read PSUM directly)
                dsb = ri_pool.tile([1, TOK], f32, name="dsb")
                nc.vector.tensor_copy(dsb[:], pv[HD:HD + 1, :])
                # stage the unnormalized ctx out of PSUM first, then reuse
                # pv's rows 0:64 for the rinv broadcast — saves a PSUM bank
                # so pv can double-buffer across heads
                cu = ri_pool.tile([64, TOK], bf16, name="cu")
                nc.vector.tensor_copy(cu[:], pv[0:HD, :])
                rinv = ri_pool.tile([1, TOK], f32, name="rinv")
                nc.vector.reciprocal_approx_fast(rinv[:], dsb[:])
                rinvb = ri_pool.tile([1, TOK], bf16, name="rinvb")
                nc.vector.tensor_copy(rinvb[:], rinv[:])
                nc.tensor.matmul(pv[0:HD, :], ones_bf[0:1, 0:64], rinvb[:],
                                 start=True, stop=True)
                cdst = cnA[0:64, :] if h == 0 else (
                    cnA[64:128, :] if h == 1 else cnB[:])
                nc.vector.tensor_mul(cdst, cu[:], pv[0:HD, :])
            # partial attn_proj for this stage's 512 tokens (bias folded
            # into xo on the host)
            for ts in range(4):
                tsl = slice(ts * 128, (ts + 1) * 128)
                po_sb = pob_pool.tile([128, C], bf16, name="po_sb")
                for c0, c1 in ((0, 512), (512, C)):
                    po = mlp_ps.tile([128, c1 - c0], f32, name="po",
                                     tag="mlp")
                    nc.tensor.matmul(po[:], cnA[:, tsl], apwA[:, c0:c1],
                                     start=True, stop=False)
                    nc.tensor.matmul(po[:], cnB[:, tsl], apwB[:, c0:c1],
                                     start=False, stop=True)
                    nc.vector.tensor_copy(po_sb[:, c0:c1], po[:])
                nc.sync.dma_start(
                    rsin[J].ap()[ts * 128:(ts + 1) * 128, :], po_sb[:])
            nc.gpsimd.collective_compute(
                "ReduceScatter", OP.add, replica_groups=groups,
                ins=[rsin[J].ap().opt()], outs=[rsout[J].ap().opt()])

        def mlp_chunk(J):
            # residual + LN2 for the 128 owned tokens of stage J
            rso = rs_pool.tile([128, C], bf16, name="rso")
            nc.sync.dma_start(rso[:], rsout[J].ap())
            nc.vector.tensor_add(x1[J][:], rso[:], xo[J][:])
            for s in range(2):
                nc.vector.bn_stats(bn2[:, J, s, :],
                                   x1[J][:, s * 384:(s + 1) * 384])
            nc.vector.bn_aggr(mv2[:, J, :], bn2[:, J, :, :])
            nc.scalar.activation(sd2[:, J, :], mv2[:, J, :],
                                 AF.Sqrt, bias=eps_t[:])
            nc.vector.reciprocal_approx_fast(rstd2[:, J, :], sd2[:, J, :])
            nc.vector.tensor_mul(nmr2[:, J:J + 1], mv2[:, J, 0:1],
                                 rstd2[:, J, 1:2])
            nc.vector.tensor_scalar(nmr2[:, J:J + 1], nmr2[:, J:J + 1], -1.0,
                                    None, OP.mult)
            h2c = rs_pool.tile([128, C], bf16, name="h2c")
            nc.scalar.activation(h2c[:], x1[J][:], AF.Identity,
                                 bias=nmr2[:, J:J + 1],
                                 scale=rstd2[:, J, 1:2])
            # transpose h2c -> h2T [128 feat, 128 tok] (6 feature chunks)
            h2T = h2t_pool.tile([128, KC, 128], bf16, name="h2T")
            for half, n in ((0, 4), (4, 2)):
                tp2 = mlp_ps.tile([128, n * 128], bf16, name="tp2",
                                  tag="mlp")
                for k in range(n):
                    nc.tensor.transpose(
                        tp2[:, k * 128:(k + 1) * 128],
                        h2c[:, (half + k) * 128:(half + k + 1) * 128],
                        ident[:])
                nc.vector.tensor_copy(
                    h2T[:, half:half + n, :].rearrange("p a b -> p (a b)"),
                    tp2[:])
            # fc, token-major: gt [128 tok, 512 ff] per ff group, gelu,
            # then transpose to ff-major g for the proj stationary
            gf = g_pool.tile([128, MT, 128], bf16, name="gf")
            gt_sb = [None] * FFC
            for i in range(FFC):
                gp = mlp_ps.tile([128, 512], f32, name="gp", tag="mlp")
                fsl = slice(i * 512, (i + 1) * 512)
                for k in range(KC):
                    nc.tensor.matmul(gp[:], h2T[:, k, :],
                                     fcw_sb[k][:, fsl],
                                     start=(k == 0), stop=(k == KC - 1))
                gt = g_pool.tile([128, 512], bf16, name="gt", bufs=3)
                if fc_zero:
                    nc.scalar.activation(gt[:], gp[:], AF.Gelu_apprx_tanh,
                                         bias=0.0)
                else:
                    nc.vector.tensor_add(gp[:], gp[:], fcb_bc[:, fsl])
                    nc.scalar.activation(gt[:], gp[:], AF.Gelu_apprx_tanh,
                                         bias=0.0)
                gt_sb[i] = gt
                if i > 0:
                    tr_g(gf, gt_sb[i - 1], i - 1)
            tr_g(gf, gt_sb[FFC - 1], FFC - 1)
            # proj + residual (+ pjb) -> out, stored per half so the first
            # half's DMA overlaps the second half's matmuls
            oo = oo_pool.tile([128, C], f32, name="oo")
            for c0, c1 in ((0, 512), (512, C)):
                pr = mlp_ps.tile([128, c1 - c0], f32, name="pr", tag="mlp")
                for m in range(MT):
                    nc.tensor.matmul(pr[:], gf[:, m, :],
                                     pjw_sb[m][:, c0:c1],
                                     start=(m == 0), stop=(m == MT - 1))
                nc.vector.tensor_add(oo[:, c0:c1], pr[:], x1[J][:, c0:c1])
                if not pjb_zero:
                    nc.vector.tensor_add(oo[:, c0:c1], oo[:, c0:c1],
                                         pjb_bc[:, c0:c1])
                nc.sync.dma_start(
                    _ap(out_d, J * 128 * C + c0, [[C, 128], [1, c1 - c0]]),
                    oo[:, c0:c1])

        def tr_g(gf, gt, i):
            tpg = mlp_ps.tile([128, 512], bf16, name="tpg", tag="mlp")
            for k in range(4):
                nc.tensor.transpose(tpg[:, k * 128:(k + 1) * 128],
                                    gt[:, k * 128:(k + 1) * 128], ident[:])
            nc.vector.tensor_copy(
                gf[:, 4 * i:4 * i + 4, :].rearrange("p a b -> p (a b)"),
                tpg[:])

        # chunk 0 consumes RS0, which can land late: defer it to stage 2's
        # head boundary so stage-2 scores fill the RS0 wait. Later chunks
        # follow the same pattern one stage after their ReduceScatter.
        attn_stage(0)
        attn_stage(1)
        attn_stage(2, mid=lambda: mlp_chunk(0))
        attn_stage(3, mid=lambda: mlp_chunk(1))
        mlp_chunk(2)
        mlp_chunk(3)
        main_es.close()
        es.close()

    nc.compile()
    return nc


def _prepare(inputs):
    """Host-side weight prep shared by kernel() and test harness."""
    x = np.asarray(inputs["x"], np.float32)
    ln1s = np.asarray(inputs["ln1_scale"], np.float32)
    ln1b = np.asarray(inputs["ln1_bias"], np.float32)
    ln2s = np.asarray(inputs["ln2_scale"], np.float32)
    ln2b = np.asarray(inputs["ln2_bias"], np.float32)
    wqkv = np.asarray(inputs["wqkv"], np.float32)          # [C,3,NH,HD]
    bqkv = np.asarray(inputs["bqkv"], np.float32)          # [3,NH,HD]
    apw = np.asarray(inputs["attn_proj_w"], np.float32)
    apb = np.asarray(inputs["attn_proj_b"], np.float32)
    fcw = np.asarray(inputs["fc_w"], np.float32)
    fcb = np.asarray(inputs["fc_b"], np.float32)
    pjw = np.asarray(inputs["proj_w"], np.float32)
    pjb = np.asarray(inputs["proj_b"], np.float32)

    # fold LN1 affine into wqkv/bqkv, LN2 affine into fcw/fcb (exact)
    wqkv_f = wqkv * ln1s[:, None, None, None]
    bqkv_f = bqkv + np.einsum("c,cshd->shd", ln1b, wqkv)
    fcw_f = fcw * ln2s[:, None]
    fcb_f = fcb + ln2b @ fcw

    fc_zero = bool(np.all(fcb_f == 0.0))
    pjb_zero = bool(np.all(pjb == 0.0))

    shared = {
        "fcw": np.ascontiguousarray(fcw_f).astype(BF),
        "pjw": pjw.astype(BF),
    }
    if not fc_zero:
        shared["fcb_bc"] = np.ascontiguousarray(
            np.broadcast_to(fcb_f[None, :], (128, FF))).astype(BF)
    if not pjb_zero:
        shared["pjb_bc"] = np.ascontiguousarray(
            np.broadcast_to(pjb[None, :], (128, C))).astype(BF)
    in_maps = []
    for core in range(N_CORES):
        b, r = divmod(core, GROUP)
        hs = slice(HPC * r, HPC * (r + 1))
        wq = wqkv_f[:, 0, hs, :].reshape(C, CROW)
        wk = wqkv_f[:, 1, hs, :].reshape(C, CROW)
        wv = wqkv_f[:, 2, hs, :].reshape(C, CROW)
        bq = bqkv_f[0, hs, :].reshape(CROW)
        bk = bqkv_f[1, hs, :].reshape(CROW)
        bv = bqkv_f[2, hs, :].reshape(CROW)
        m = dict(shared)
        # packed: [qA(128) | kA(128) | q_h2(64)+k_h2(64) | v(192)]
        m["wqkv"] = np.ascontiguousarray(np.concatenate(
            [wq[:, :128], wk[:, :128], wq[:, 128:], wk[:, 128:], wv],
            axis=1)).astype(BF)
        m["bqA"] = np.ascontiguousarray(bq[:128]).astype(np.float32)
        m["bkA"] = np.ascontiguousarray(bk[:128]).astype(np.float32)
        m["bqkB"] = np.ascontiguousarray(
            np.concatenate([bq[128:], bk[128:]])).astype(np.float32)
        m["bvb"] = np.ascontiguousarray(
            np.broadcast_to(bv[None, :], (128, CROW))).astype(BF)
        m["apw"] = np.ascontiguousarray(
            apw[CROW * r:CROW * (r + 1), :]).astype(BF)
        m["xb"] = np.ascontiguousarray(x[b]).astype(BF)
        # attn_proj bias folded into the residual input (exact)
        m["xo"] = np.ascontiguousarray(
            x[b].reshape(NT, 128, C)[r::GROUP] + apb[None, None, :])
        in_maps.append(m)
    return in_maps, fc_zero, pjb_zero


def kernel(x, mask, ln1_scale, ln1_bias, wqkv, bqkv, attn_proj_w, attn_proj_b,
           ln2_scale, ln2_bias, fc_w, fc_b, proj_w, proj_b):
    in_maps, fc_zero, pjb_zero = _prepare(dict(
        x=x, ln1_scale=ln1_scale, ln1_bias=ln1_bias, wqkv=wqkv, bqkv=bqkv,
        attn_proj_w=attn_proj_w, attn_proj_b=attn_proj_b,
        ln2_scale=ln2_scale, ln2_bias=ln2_bias, fc_w=fc_w, fc_b=fc_b,
        proj_w=proj_w, proj_b=proj_b))
    key = (fc_zero, pjb_zero)
    if _CACHE.get("key") != key:
        _CACHE["nc"] = _build(fc_zero, pjb_zero)
        _CACHE["key"] = key
    nc = _CACHE["nc"]
    res = run_bass_kernel_spmd(nc, in_maps, list(range(N_CORES)))
    _CACHE["last_result"] = res
    return assemble(res.results)


def assemble(results):
    """Assemble per-core 'out' results into the full [B, T, C] output."""
    out = np.empty((B, T, C), dtype=np.float32)
    for core in range(N_CORES):
        b, r = divmod(core, GROUP)
        out[b].reshape(NT, 128, C)[r::GROUP] = results[core]["out"]
    return out
